# revision 40
# baseline (speedup 1.0000x reference)
"""Trainium2 Bass kernel for nn_LocalHiddenPositiveProjection.

Computation (per batch b):
  a = mean_h attn[b, :, 1:, 1:]                  # [N, N], N = 3136
  a = (a - rowmin) / (rowmax - rowmin)           # per-row min-max norm
  a[a > rowquantile(a, 0.99)] = 0                # zero top-32 per row (exact)
  mixed = a @ code[b].reshape(C, N).T / N        # [N, C] -> [C, N]
  out = W2 @ relu(W1 @ mixed + b1) + b2          # 1x1 conv head

Key reformulation: min-max norm is invariant to the head-mean (affine), so we
work on s = sum over heads. The 0.99-quantile cut over 3136 elements zeroes
exactly the elements > q, and q lies strictly between the 32nd and 33rd
largest (verified: no fp32 ties for this generator), i.e. the top-32 per row.
We extract top-32 with 4 rounds of vector.max + match_replace(-1e30); a single
fused ScalarE activation relu(s*scale + bias) then produces the masked,
normalized, 1/N-scaled weights (the -1e30 holes relu to 0).

Engine balance (cost-model): the DMA of attn (59MB/core at ~360GB/s) is the
roofline at ~27us per 128-row tile. Most of the 6-head sum runs inside the
DMA engines (SWDGE CCE fp32 accumulate, bitwise-exact; split into <=1568-
element descriptors because CCE corrupts past 2048 elements): three chains
of 2 per tile, merged by two Pool adds. All attn loads go through the
gpsimd (SWDGE) queue so the DMA device services them in exact tile order.
DVE carries the top-32 rounds (8 scans ~26.6us/tile) plus a prefix-min scan
for the row minimum; normalize+mask runs on ScalarE (chunked so PE
transposes start early); transposes + matmuls (bf16 weights/code) on PE.
The 16-row runt tile is processed in a packed [128, 392] layout (8 segments
per row across partitions -> 8x narrower scans: per-segment top-32
candidates, regrouped, reduced to the exact row top-32, masked via
match_replace broadcast), in phases interleaved between the last full tiles.

Sharding: 8 cores, data-parallel over (batch, query-row quarter):
core c handles batch c//4, rows (c%4)*784 ... +784.
"""

import os
from contextlib import ExitStack

import ml_dtypes
import numpy as np

import concourse.bass as bass
import concourse.mybir as mybir
import concourse.tile as tile
from concourse import bacc
from concourse.bass_utils import run_bass_kernel_spmd
from concourse.masks import make_identity

F32 = mybir.dt.float32
F32R = mybir.dt.float32r
BF16 = mybir.dt.bfloat16
AX = mybir.AxisListType
ALU = mybir.AluOpType
ACTF = mybir.ActivationFunctionType

B, HEADS, DIM, SZ = 2, 6, 384, 56
N = SZ * SZ            # 3136
NP1 = N + 1            # 3137
NCORES = 8
ROWS_PER_CORE = (B * N) // NCORES   # 784
TILE_ROWS = 128
NEG_HUGE = -1.0e30

# row tiles: 6 x 128 + 1 x 16
ROW_TILES = []
_r = 0
while _r < ROWS_PER_CORE:
    ROW_TILES.append((_r, min(TILE_ROWS, ROWS_PER_CORE - _r)))
    _r += TILE_ROWS

# transpose / contraction m-chunks of 128 (24 full + 1 of 64)
K_CHUNKS = [(i * 128, min(128, N - i * 128)) for i in range((N + 127) // 128)]
NKC = len(K_CHUNKS)  # 25
# groups of up to 4 transpose chunks per PSUM bank
TR_GROUPS = [list(range(g, min(g + 4, NKC))) for g in range(0, NKC, 4)]

NOC = DIM // 128  # 3 chunks of 128 over the channel dim

# runt tile (16 rows) packing: 8 segments of 392 across the partition dim
RUNT_SEG = 8
RUNT_W = N // RUNT_SEG  # 392

# head-sum via one DMA-accumulate chain over all six heads (CCE fp32 add,
# bitwise-exact left-to-right order)
CHAIN = (0, 1, 2, 3, 4, 5)


def emit_kernel(tc, attn_s, attn_r, code_t, w1t, b1, w2t, b2, out_s, ctx):
    nc = tc.nc

    singles = ctx.enter_context(tc.tile_pool(name="singles", bufs=1))

    ident = singles.tile([128, 128], F32, tag="ident")
    make_identity(nc, ident)

    # biases as per-partition [128, 1] columns (chunk i in column i)
    # setup DMAs go on the Activation HWDGE queue so they never block the
    # per-tile attn chain loads on the SP queue.
    b1_sb = singles.tile([128, NOC], F32, tag="b1")
    b2_sb = singles.tile([128, NOC], F32, tag="b2")
    for i in range(NOC):
        nc.scalar.dma_start(out=b1_sb[:, i : i + 1], in_=b1[i * 128 : (i + 1) * 128])
        nc.scalar.dma_start(out=b2_sb[:, i : i + 1], in_=b2[i * 128 : (i + 1) * 128])

    # code^T: [m (25 chunks of <=128 partitions), c (384)] — host pre-transposed,
    # loaded directly chunk by chunk
    codefT = singles.tile([128, NKC, DIM], BF16, tag="codefT")
    for j, (m0, mw) in enumerate(K_CHUNKS):
        nc.scalar.dma_start(out=codefT[:mw, j, :], in_=code_t[m0 : m0 + mw, :])
    # W1^T / W2^T: [c-chunk j partitions, o (384)] — host pre-transposed
    w1T = singles.tile([128, NOC, DIM], F32R, tag="w1T")
    w2T = singles.tile([128, NOC, DIM], F32R, tag="w2T")
    for wsrc, wdst in ((w1t, w1T), (w2t, w2T)):
        for j in range(NOC):
            nc.scalar.dma_start(
                out=wdst[:, j, :], in_=wsrc[j * 128 : (j + 1) * 128, :]
            )

    heads = ctx.enter_context(tc.tile_pool(name="heads", bufs=11))
    wt_pool = ctx.enter_context(tc.tile_pool(name="wt", bufs=2))
    proj = ctx.enter_context(tc.tile_pool(name="proj", bufs=2))
    smalls = ctx.enter_context(tc.tile_pool(name="smalls", bufs=2))
    outp = ctx.enter_context(tc.tile_pool(name="outp", bufs=3))
    runt_pool = ctx.enter_context(tc.tile_pool(name="runt", bufs=4))

    ps_tr = ctx.enter_context(tc.tile_pool(name="ps_tr", bufs=2, space="PSUM"))
    ps_mix = ctx.enter_context(tc.tile_pool(name="ps_mix", bufs=2, space="PSUM"))
    ps_proj = ctx.enter_context(tc.tile_pool(name="ps_proj", bufs=2, space="PSUM"))

    # ---- per-tile: head-sum DMA chains + compute, emitted tile-serial so
    # every in-order engine queue sees its work in dependency order ----
    # tile 0: six plain loads (first in the SP queue -> the DMA device
    # delivers them back-to-back, fastest pipeline fill) + an add tree.
    # others: 3 chains of 2 - every CCE accumulate waits only on a dep-free
    # plain load, so the Pool sequencer never stalls. Accumulating DMAs are
    # split into half rows: the CCE path corrupts descriptors longer than
    # 2048 elements (hardware-verified), halves of 1568 are exact.
    ACC_CHUNKS = [(0, N // 2), (N // 2, N // 2)]

    def tile_chain_shapes(tidx):
        if tidx == 0:
            return [(h,) for h in range(6)]
        return [(0, 1), (2, 3), (4, 5)]

    FMAX = float(np.finfo(np.float32).max)
    NTILES = len(ROW_TILES)

    def emit_smalls_and_tail(s_sb, smin, vals, row0, rows):
        # scale = 1/((smax-smin)*N);  bias = -smin*scale
        rng = smalls.tile([TILE_ROWS, 1], F32, tag="rng")
        nc.vector.tensor_sub(rng[:rows, :], vals[:rows, 0:1], smin)
        inv = smalls.tile([TILE_ROWS, 1], F32, tag="inv")
        nc.vector.reciprocal(inv[:rows, :], rng[:rows, :])
        scale = smalls.tile([TILE_ROWS, 1], F32, tag="scale")
        nc.vector.tensor_scalar_mul(scale[:rows, :], inv[:rows, :], 1.0 / N)
        nbias = smalls.tile([TILE_ROWS, 1], F32, tag="nbias")
        nc.vector.tensor_mul(nbias[:rows, :], smin, scale[:rows, :])
        nc.vector.tensor_scalar_mul(nbias[:rows, :], nbias[:rows, :], -1.0)

        # fused normalize + mask: w = relu(s*scale + bias), chunked per
        # transpose group so transposes start before the whole row is done
        wT = wt_pool.tile([128, NKC, TILE_ROWS], BF16, tag="wT")
        for grp in TR_GROUPS:
            a0 = K_CHUNKS[grp[0]][0]
            a1 = K_CHUNKS[grp[-1]][0] + K_CHUNKS[grp[-1]][1]
            nc.scalar.activation(
                out=s_sb[:rows, a0:a1],
                in_=s_sb[:rows, a0:a1],
                func=ACTF.Relu,
                bias=nbias[:rows, :],
                scale=scale[:rows, :],
            )
            tp = ps_tr.tile([128, 4, TILE_ROWS], F32, tag="tr")
            for k, j in enumerate(grp):
                m0, mw = K_CHUNKS[j]
                nc.tensor.transpose(
                    tp[:mw, k, :rows], s_sb[:rows, m0 : m0 + mw],
                    ident[:rows, :rows],
                )
            gw = 128 if len(grp) == 4 else K_CHUNKS[grp[0]][1]
            nc.scalar.copy(
                out=wT[:gw, grp[0] : grp[0] + len(grp), :rows],
                in_=tp[:gw, : len(grp), :rows],
            )
        mixp = ps_mix.tile([TILE_ROWS, DIM], F32, tag="mix")
        for j, (m0, mw) in enumerate(K_CHUNKS):
            nc.tensor.matmul(
                mixp[:rows, :],
                lhsT=wT[:mw, j, :rows],
                rhs=codefT[:mw, j, :],
                start=(j == 0),
                stop=(j == NKC - 1),
            )
        mix_sb = proj.tile([TILE_ROWS, DIM], F32, tag="mix_sb")
        nc.scalar.copy(out=mix_sb[:rows, :], in_=mixp[:rows, :])

        # mixed^T: [c, n]
        tpm = ps_tr.tile([128, 4, TILE_ROWS], F32, tag="tr")
        for i in range(NOC):
            nc.tensor.transpose(
                tpm[:, i, :rows], mix_sb[:rows, i * 128 : (i + 1) * 128],
                ident[:rows, :rows],
            )
        mixT = proj.tile([128, NOC, TILE_ROWS], F32R, tag="mixT")
        nc.scalar.copy(out=mixT[:, :, :rows], in_=tpm[:, :NOC, :rows])

        # h = relu(W1 @ mixed + b1)
        h_sb = proj.tile([128, NOC, TILE_ROWS], F32R, tag="h_sb")
        for i in range(NOC):
            hp = ps_proj.tile([128, TILE_ROWS], F32, tag="pp")
            for j in range(NOC):
                nc.tensor.matmul(
                    hp[:, :rows],
                    lhsT=w1T[:, j, i * 128 : (i + 1) * 128],
                    rhs=mixT[:, j, :rows],
                    start=(j == 0),
                    stop=(j == NOC - 1),
                )
            nc.scalar.activation(
                out=h_sb[:, i, :rows], in_=hp[:, :rows], func=ACTF.Relu,
                bias=b1_sb[:, i : i + 1], scale=1.0,
            )

        # out = W2 @ h + b2
        for i in range(NOC):
            op = ps_proj.tile([128, TILE_ROWS], F32, tag="pp")
            for j in range(NOC):
                nc.tensor.matmul(
                    op[:, :rows],
                    lhsT=w2T[:, j, i * 128 : (i + 1) * 128],
                    rhs=h_sb[:, j, :rows],
                    start=(j == 0),
                    stop=(j == NOC - 1),
                )
            ob = outp.tile([128, TILE_ROWS], F32, tag="ob")
            nc.scalar.activation(
                out=ob[:, :rows], in_=op[:, :rows], func=ACTF.Identity,
                bias=b2_sb[:, i : i + 1], scale=1.0,
            )
            nc.sync.dma_start(
                out=out_s[i * 128 : (i + 1) * 128, row0 : row0 + rows],
                in_=ob[:, :rows],
            )

    # ---- runt tile (16 rows): processed in a packed [128, 392] layout
    # (partition = seg*16 + r) so every DVE scan is 8x narrower. Split into
    # phases interleaved between the first full tiles so each phase's small-
    # DMA latencies hide behind a full tile of streaming. ----
    runt_state = {}
    runt_row0, runt_rows = ROW_TILES[-1]

    def runt_phase0():
        rows = runt_rows
        pk = []
        for chain in [(0, 1), (2, 3), (4, 5)]:
            t = runt_pool.tile([TILE_ROWS, RUNT_W], F32, tag="pk")
            nc.gpsimd.dma_start(out=t, in_=attn_r[chain[0]])
            pk.append(t)
        for ci, chain in enumerate([(0, 1), (2, 3), (4, 5)]):
            nc.gpsimd.dma_start(
                out=pk[ci], in_=attn_r[chain[1]], accum_op=ALU.add
            )
        ps_ = pk[0]
        nc.gpsimd.tensor_add(pk[1][:, :], pk[1][:, :], pk[2][:, :])
        nc.gpsimd.tensor_add(ps_[:, :], ps_[:, :], pk[1][:, :])
        # per-segment min scan
        nc.vector.tensor_tensor_scan(
            out=pk[1][:, :], data0=ps_[:, :], data1=ps_[:, :],
            initial=FMAX, op0=ALU.min, op1=ALU.min,
        )
        minT = smalls.tile([TILE_ROWS, 8], F32, tag="minT")
        for seg in range(RUNT_SEG):
            nc.sync.dma_start(
                out=minT[0:rows, seg : seg + 1],
                in_=pk[1][seg * rows : (seg + 1) * rows, RUNT_W - 1 : RUNT_W],
            )
        # per-segment top-32 candidates (on a copy)
        sc = runt_pool.tile([TILE_ROWS, RUNT_W], F32, tag="pk")
        nc.scalar.copy(out=sc[:, :], in_=ps_[:, :])
        svals = smalls.tile([TILE_ROWS, 32], F32, tag="svals")
        for r in range(4):
            nc.vector.max(out=svals[:, r * 8 : (r + 1) * 8], in_=sc[:, :])
            nc.vector.match_replace(
                out=sc[:, :],
                in_to_replace=svals[:, r * 8 : (r + 1) * 8],
                in_values=sc[:, :],
                imm_value=NEG_HUGE,
            )
        runt_state.update(ps_=ps_, minT=minT, svals=svals)

    def runt_phase1():
        rows = runt_rows
        minT, svals = runt_state["minT"], runt_state["svals"]
        smin_t = smalls.tile([TILE_ROWS, 1], F32, tag="smin_r")
        nc.vector.tensor_reduce(
            out=smin_t[:rows, :], in_=minT[:rows, :], axis=AX.X, op=ALU.min
        )
        cand = smalls.tile([TILE_ROWS, RUNT_SEG * 32], F32, tag="cand")
        for seg in range(RUNT_SEG):
            nc.sync.dma_start(
                out=cand[0:rows, seg * 32 : (seg + 1) * 32],
                in_=svals[seg * rows : (seg + 1) * rows, 0:32],
            )
        vals = smalls.tile([TILE_ROWS, 32], F32, tag="vals_r")
        for r in range(4):
            nc.vector.max(out=vals[:rows, r * 8 : (r + 1) * 8], in_=cand[:rows, :])
            nc.vector.match_replace(
                out=cand[:rows, :],
                in_to_replace=vals[:rows, r * 8 : (r + 1) * 8],
                in_values=cand[:rows, :],
                imm_value=NEG_HUGE,
            )
        runt_state.update(smin_t=smin_t, vals=vals)

    def runt_phase2():
        rows = runt_rows
        ps_, vals = runt_state["ps_"], runt_state["vals"]
        vals_b = smalls.tile([TILE_ROWS, 32], F32, tag="valsb")
        for seg in range(RUNT_SEG):
            nc.sync.dma_start(
                out=vals_b[seg * rows : (seg + 1) * rows, :],
                in_=vals[0:rows, 0:32],
            )
        for r in range(4):
            nc.vector.match_replace(
                out=ps_[:, :],
                in_to_replace=vals_b[:, r * 8 : (r + 1) * 8],
                in_values=ps_[:, :],
                imm_value=NEG_HUGE,
            )
        s_sb = heads.tile([TILE_ROWS, N], F32, tag="chain")
        for seg in range(RUNT_SEG):
            nc.sync.dma_start(
                out=s_sb[0:rows, seg * RUNT_W : (seg + 1) * RUNT_W],
                in_=ps_[seg * rows : (seg + 1) * rows, :],
            )
        runt_state.update(s_sb=s_sb)

    def runt_phase3():
        emit_smalls_and_tail(
            runt_state["s_sb"], runt_state["smin_t"][:runt_rows, :],
            runt_state["vals"], runt_row0, runt_rows,
        )

    # phase k runs after full tile PHASE_AFTER[k]; loads late enough that
    # the ramp stays pristine, early enough that each phase's small-DMA
    # latencies hide behind a full tile of rounds
    runt_phases = [runt_phase0, runt_phase1, runt_phase2, runt_phase3]
    PHASE_AFTER = {2: runt_phase0, 3: runt_phase1, 4: runt_phase2}

    for tidx in range(NTILES - 1):
        row0, rows = ROW_TILES[tidx]
        if True:
            chain_t = []
            for chain in tile_chain_shapes(tidx):
                t = heads.tile([TILE_ROWS, N], F32, tag="chain")
                nc.gpsimd.dma_start(
                    out=t[:rows, :],
                    in_=attn_s[chain[0], row0 : row0 + rows, 1:NP1],
                )
                chain_t.append(t)
            for ci, chain in enumerate(tile_chain_shapes(tidx)):
                for h in chain[1:]:
                    for c0, cw in ACC_CHUNKS:
                        nc.gpsimd.dma_start(
                            out=chain_t[ci][:rows, c0 : c0 + cw],
                            in_=attn_s[h, row0 : row0 + rows, 1 + c0 : 1 + c0 + cw],
                            accum_op=ALU.add,
                        )

            s_sb = chain_t[0]
            if len(chain_t) == 3:
                # merge the pair-sums on Pool, split by halves so the half-0
                # merges overlap the half-1 accumulate transfers (early
                # s-readiness with zero DVE cost)
                for c0, cw in ACC_CHUNKS:
                    nc.gpsimd.tensor_add(
                        chain_t[1][:rows, c0 : c0 + cw],
                        chain_t[1][:rows, c0 : c0 + cw],
                        chain_t[2][:rows, c0 : c0 + cw],
                    )
                    nc.gpsimd.tensor_add(
                        s_sb[:rows, c0 : c0 + cw],
                        s_sb[:rows, c0 : c0 + cw],
                        chain_t[1][:rows, c0 : c0 + cw],
                    )
            else:
                # ramp tile: six plain loads merged with an all-DVE add tree,
                # split by halves so adds start as half-loads land
                s_sb = chain_t[4]
                for c0, cw in ACC_CHUNKS:
                    nc.vector.tensor_add(
                        chain_t[1][:rows, c0 : c0 + cw],
                        chain_t[0][:rows, c0 : c0 + cw],
                        chain_t[1][:rows, c0 : c0 + cw],
                    )
                    nc.vector.tensor_add(
                        chain_t[2][:rows, c0 : c0 + cw],
                        chain_t[2][:rows, c0 : c0 + cw],
                        chain_t[3][:rows, c0 : c0 + cw],
                    )
                    nc.vector.tensor_add(
                        chain_t[4][:rows, c0 : c0 + cw],
                        chain_t[4][:rows, c0 : c0 + cw],
                        chain_t[5][:rows, c0 : c0 + cw],
                    )
                    nc.vector.tensor_add(
                        chain_t[1][:rows, c0 : c0 + cw],
                        chain_t[1][:rows, c0 : c0 + cw],
                        chain_t[2][:rows, c0 : c0 + cw],
                    )
                    nc.vector.tensor_add(
                        s_sb[:rows, c0 : c0 + cw],
                        s_sb[:rows, c0 : c0 + cw],
                        chain_t[1][:rows, c0 : c0 + cw],
                    )
            # ---- row min via prefix-min scan (last column = row min) ----
            scanout = chain_t[1]
            nc.vector.tensor_tensor_scan(
                out=scanout[:rows, :],
                data0=s_sb[:rows, :],
                data1=s_sb[:rows, :],
                initial=FMAX,
                op0=ALU.min,
                op1=ALU.min,
            )
            smin = scanout[:rows, N - 1 : N]
            vals = smalls.tile([TILE_ROWS, 32], F32, tag="vals")
            for r in range(4):
                nc.vector.max(
                    out=vals[:rows, r * 8 : (r + 1) * 8], in_=s_sb[:rows, :]
                )
                nc.vector.match_replace(
                    out=s_sb[:rows, :],
                    in_to_replace=vals[:rows, r * 8 : (r + 1) * 8],
                    in_values=s_sb[:rows, :],
                    imm_value=NEG_HUGE,
                )
        else:
            # ---- runt tile (16 rows): processed in a packed [128, 392]
            # layout (partition = seg*16 + r, seg in 0..7) so every DVE scan
            # is 8x narrower. Loads come from the host-pre-packed attn_r. ----
            pk = []
            for chain in [(0, 1), (2, 3), (4, 5)]:
                t = runt_pool.tile([TILE_ROWS, RUNT_W], F32, tag="pk")
                nc.gpsimd.dma_start(out=t, in_=attn_r[chain[0]])
                pk.append(t)
            for ci, chain in enumerate([(0, 1), (2, 3), (4, 5)]):
                nc.gpsimd.dma_start(
                    out=pk[ci], in_=attn_r[chain[1]], accum_op=ALU.add
                )
            ps_ = pk[0]
            nc.gpsimd.tensor_add(pk[1][:, :], pk[1][:, :], pk[2][:, :])
            nc.gpsimd.tensor_add(ps_[:, :], ps_[:, :], pk[1][:, :])
            # per-segment min scan -> row min
            nc.vector.tensor_tensor_scan(
                out=pk[1][:, :], data0=ps_[:, :], data1=ps_[:, :],
                initial=FMAX, op0=ALU.min, op1=ALU.min,
            )
            minT = smalls.tile([TILE_ROWS, 8], F32, tag="minT")
            for seg in range(RUNT_SEG):
                nc.sync.dma_start(
                    out=minT[0:rows, seg : seg + 1],
                    in_=pk[1][seg * rows : (seg + 1) * rows, RUNT_W - 1 : RUNT_W],
                )
            smin_t = smalls.tile([TILE_ROWS, 1], F32, tag="smin")
            nc.vector.tensor_reduce(
                out=smin_t[:rows, :], in_=minT[:rows, :], axis=AX.X, op=ALU.min
            )
            smin = smin_t[:rows, :]
            # per-segment top-32 candidates (on a copy), regroup per row
            sc = runt_pool.tile([TILE_ROWS, RUNT_W], F32, tag="pk")
            nc.scalar.copy(out=sc[:, :], in_=ps_[:, :])
            svals = smalls.tile([TILE_ROWS, 32], F32, tag="svals")
            for r in range(4):
                nc.vector.max(out=svals[:, r * 8 : (r + 1) * 8], in_=sc[:, :])
                nc.vector.match_replace(
                    out=sc[:, :],
                    in_to_replace=svals[:, r * 8 : (r + 1) * 8],
                    in_values=sc[:, :],
                    imm_value=NEG_HUGE,
                )
            cand = smalls.tile([TILE_ROWS, RUNT_SEG * 32], F32, tag="cand")
            for seg in range(RUNT_SEG):
                nc.sync.dma_start(
                    out=cand[0:rows, seg * 32 : (seg + 1) * 32],
                    in_=svals[seg * rows : (seg + 1) * rows, 0:32],
                )
            # exact row top-32 from the candidates
            vals = smalls.tile([TILE_ROWS, 32], F32, tag="vals")
            for r in range(4):
                nc.vector.max(out=vals[:rows, r * 8 : (r + 1) * 8], in_=cand[:rows, :])
                nc.vector.match_replace(
                    out=cand[:rows, :],
                    in_to_replace=vals[:rows, r * 8 : (r + 1) * 8],
                    in_values=cand[:rows, :],
                    imm_value=NEG_HUGE,
                )
            # broadcast row top-32 to every segment, mask the packed s
            vals_b = smalls.tile([TILE_ROWS, 32], F32, tag="valsb")
            for seg in range(RUNT_SEG):
                nc.sync.dma_start(
                    out=vals_b[seg * rows : (seg + 1) * rows, :],
                    in_=vals[0:rows, 0:32],
                )
            for r in range(4):
                nc.vector.match_replace(
                    out=ps_[:, :],
                    in_to_replace=vals_b[:, r * 8 : (r + 1) * 8],
                    in_values=ps_[:, :],
                    imm_value=NEG_HUGE,
                )
            # unpack the masked packed s into row-major [16, 3136]
            s_sb = heads.tile([TILE_ROWS, N], F32, tag="chain")
            for seg in range(RUNT_SEG):
                nc.sync.dma_start(
                    out=s_sb[0:rows, seg * RUNT_W : (seg + 1) * RUNT_W],
                    in_=ps_[seg * rows : (seg + 1) * rows, :],
                )
        # scale = 1/((smax-smin)*N);  bias = -smin*scale
        rng = smalls.tile([TILE_ROWS, 1], F32, tag="rng")
        nc.vector.tensor_sub(rng[:rows, :], vals[:rows, 0:1], smin[:rows, :])
        inv = smalls.tile([TILE_ROWS, 1], F32, tag="inv")
        nc.vector.reciprocal(inv[:rows, :], rng[:rows, :])
        scale = smalls.tile([TILE_ROWS, 1], F32, tag="scale")
        nc.vector.tensor_scalar_mul(scale[:rows, :], inv[:rows, :], 1.0 / N)
        nbias = smalls.tile([TILE_ROWS, 1], F32, tag="nbias")
        nc.vector.tensor_mul(nbias[:rows, :], smin[:rows, :], scale[:rows, :])
        nc.vector.tensor_scalar_mul(nbias[:rows, :], nbias[:rows, :], -1.0)

        # ---- fused normalize + mask: w = relu(s*scale + bias), chunked per
        # transpose group so the transposes can start before the whole row is
        # normalized ----
        wT = wt_pool.tile([128, NKC, TILE_ROWS], BF16, tag="wT")
        for grp in TR_GROUPS:
            a0 = K_CHUNKS[grp[0]][0]
            a1 = K_CHUNKS[grp[-1]][0] + K_CHUNKS[grp[-1]][1]
            nc.scalar.activation(
                out=s_sb[:rows, a0:a1],
                in_=s_sb[:rows, a0:a1],
                func=ACTF.Relu,
                bias=nbias[:rows, :],
                scale=scale[:rows, :],
            )
            tp = ps_tr.tile([128, 4, TILE_ROWS], F32, tag="tr")
            for k, j in enumerate(grp):
                m0, mw = K_CHUNKS[j]
                nc.tensor.transpose(
                    tp[:mw, k, :rows], s_sb[:rows, m0 : m0 + mw], ident[:rows, :rows]
                )
            gw = 128 if len(grp) == 4 else K_CHUNKS[grp[0]][1]
            nc.scalar.copy(
                out=wT[:gw, grp[0] : grp[0] + len(grp), :rows],
                in_=tp[:gw, : len(grp), :rows],
            )
        mixp = ps_mix.tile([TILE_ROWS, DIM], F32, tag="mix")
        for j, (m0, mw) in enumerate(K_CHUNKS):
            nc.tensor.matmul(
                mixp[:rows, :],
                lhsT=wT[:mw, j, :rows],
                rhs=codefT[:mw, j, :],
                start=(j == 0),
                stop=(j == NKC - 1),
            )
        mix_sb = proj.tile([TILE_ROWS, DIM], F32, tag="mix_sb")
        nc.scalar.copy(out=mix_sb[:rows, :], in_=mixp[:rows, :])

        # ---- mixed^T: [c, n] ----
        tpm = ps_tr.tile([128, 4, TILE_ROWS], F32, tag="tr")
        for i in range(NOC):
            nc.tensor.transpose(
                tpm[:, i, :rows], mix_sb[:rows, i * 128 : (i + 1) * 128],
                ident[:rows, :rows],
            )
        mixT = proj.tile([128, NOC, TILE_ROWS], F32R, tag="mixT")
        nc.scalar.copy(out=mixT[:, :, :rows], in_=tpm[:, :NOC, :rows])

        # ---- h = relu(W1 @ mixed + b1) ----
        h_sb = proj.tile([128, NOC, TILE_ROWS], F32R, tag="h_sb")
        for i in range(NOC):
            hp = ps_proj.tile([128, TILE_ROWS], F32, tag="pp")
            for j in range(NOC):
                nc.tensor.matmul(
                    hp[:, :rows],
                    lhsT=w1T[:, j, i * 128 : (i + 1) * 128],
                    rhs=mixT[:, j, :rows],
                    start=(j == 0),
                    stop=(j == NOC - 1),
                )
            nc.scalar.activation(
                out=h_sb[:, i, :rows], in_=hp[:, :rows], func=ACTF.Relu,
                bias=b1_sb[:, i : i + 1], scale=1.0,
            )

        # ---- out = W2 @ h + b2 ----
        for i in range(NOC):
            op = ps_proj.tile([128, TILE_ROWS], F32, tag="pp")
            for j in range(NOC):
                nc.tensor.matmul(
                    op[:, :rows],
                    lhsT=w2T[:, j, i * 128 : (i + 1) * 128],
                    rhs=h_sb[:, j, :rows],
                    start=(j == 0),
                    stop=(j == NOC - 1),
                )
            ob = outp.tile([128, TILE_ROWS], F32, tag="ob")
            nc.scalar.activation(
                out=ob[:, :rows], in_=op[:, :rows], func=ACTF.Identity,
                bias=b2_sb[:, i : i + 1], scale=1.0,
            )
            nc.sync.dma_start(
                out=out_s[i * 128 : (i + 1) * 128, row0 : row0 + rows],
                in_=ob[:, :rows],
            )


def build_program():
    nc = bacc.Bacc("TRN2", target_bir_lowering=False, debug=False, num_swdge_queues=4)
    attn_s = nc.dram_tensor("attn_s", [HEADS, ROWS_PER_CORE, NP1], F32, kind="ExternalInput")
    attn_r = nc.dram_tensor("attn_r", [HEADS, TILE_ROWS, RUNT_W], F32, kind="ExternalInput")
    code_t = nc.dram_tensor("code_t", [N, DIM], BF16, kind="ExternalInput")
    w1t = nc.dram_tensor("w1t", [DIM, DIM], F32R, kind="ExternalInput")
    b1 = nc.dram_tensor("b1", [DIM], F32, kind="ExternalInput")
    w2t = nc.dram_tensor("w2t", [DIM, DIM], F32R, kind="ExternalInput")
    b2 = nc.dram_tensor("b2", [DIM], F32, kind="ExternalInput")
    out_s = nc.dram_tensor("out_s", [DIM, ROWS_PER_CORE], F32, kind="ExternalOutput")

    with tile.TileContext(nc) as tc, ExitStack() as ctx:
        emit_kernel(
            tc, attn_s.ap(), attn_r.ap(), code_t.ap(), w1t.ap(), b1.ap(),
            w2t.ap(), b2.ap(), out_s.ap(), ctx,
        )
    nc.compile()
    return nc


_NC_CACHE = None
LAST_EXEC_NS = None


def _get_program():
    global _NC_CACHE
    if _NC_CACHE is None:
        _NC_CACHE = build_program()
    return _NC_CACHE


def make_in_maps(code, attn, W1, b1, W2, b2):
    code = np.asarray(code, dtype=np.float32)
    attn = np.asarray(attn, dtype=np.float32)
    in_maps = []
    for c in range(NCORES):
        b = c // (NCORES // B)
        n0 = (c % (NCORES // B)) * ROWS_PER_CORE
        runt0 = ROW_TILES[-1][0]
        nrunt = ROW_TILES[-1][1]
        attn_runt = (
            attn[b, :, 1 + n0 + runt0 : 1 + n0 + runt0 + nrunt, 1:]
            .reshape(HEADS, nrunt, RUNT_SEG, RUNT_W)
            .transpose(0, 2, 1, 3)
            .reshape(HEADS, RUNT_SEG * nrunt, RUNT_W)
        )
        attn_runt_full = np.zeros((HEADS, TILE_ROWS, RUNT_W), np.float32)
        attn_runt_full[:, : RUNT_SEG * nrunt, :] = attn_runt
        in_maps.append(
            {
                "attn_s": np.ascontiguousarray(
                    attn[b, :, 1 + n0 : 1 + n0 + ROWS_PER_CORE, :]
                ),
                "attn_r": attn_runt_full,
                "code_t": np.ascontiguousarray(
                    code[b].reshape(DIM, N).T
                ).astype(ml_dtypes.bfloat16),
                "w1t": np.ascontiguousarray(np.asarray(W1, dtype=np.float32).T),
                "b1": np.asarray(b1, dtype=np.float32),
                "w2t": np.ascontiguousarray(np.asarray(W2, dtype=np.float32).T),
                "b2": np.asarray(b2, dtype=np.float32),
            }
        )
    return in_maps


def kernel(code, attn, W1, b1, W2, b2):
    nc = _get_program()
    in_maps = make_in_maps(code, attn, W1, b1, W2, b2)
    trace = bool(int(os.environ.get("KERNEL_TRACE", "0")))
    res = run_bass_kernel_spmd(nc, in_maps, list(range(NCORES)), trace=trace)
    global LAST_EXEC_NS
    LAST_EXEC_NS = res.exec_time_ns
    if res.exec_time_ns is not None:
        print(f"HW exec time: {res.exec_time_ns} ns")
    out = np.empty((B, DIM, N), np.float32)
    for c in range(NCORES):
        b = c // (NCORES // B)
        n0 = (c % (NCORES // B)) * ROWS_PER_CORE
        out[b, :, n0 : n0 + ROWS_PER_CORE] = res.results[c]["out_s"]
    return out.reshape(B, DIM, SZ, SZ)

